# revision 44
# baseline (speedup 1.0000x reference)
"""PRXAttention TRN2 kernel: 8-core SPMD (2 batches x 4 head-groups).

Per core (b, g): project q/k/v for 4 heads (img) + k/v (txt), RMSNorm via
PE ones-matmul partition reduction, RoPE with host-prepared pair-deinterleaved
tables (g_q/g_k folded in), softmax without max-subtraction (scores bounded),
masking via host-side zeroing of masked encoder tokens + Z correction,
partial out-projection for the 4 heads; host sums the 4 partials per batch.

Engine plan: PE is the pacer everywhere. Input DMAs are split across the SP
and Act HWDGE queues and prefetched a phase early. In attention, the softmax
Z chain and the out-projection matmuls are deferred into a pending-work queue
drained one item per key-loop iteration of the NEXT head, so the PE stream
always has ready work while Act does exp.
"""

import numpy as np
import ml_dtypes

bf16 = ml_dtypes.bfloat16

B, L_IMG, L_TXT = 2, 2048, 2048 // 4
D, H, DH = 2048, 16, 128
HPC = 4                      # heads per core
NCORES = 8
EPS = 1e-6
SM_SCALE = 1.0 / float(np.sqrt(DH))
NDT = D // 128               # 16 d-model tiles
NLC = L_IMG // 512           # 4 img l-chunks of 512
NKT_TXT = L_TXT // 128       # 4 txt key tiles
NKT = NKT_TXT + L_IMG // 128  # 20 key tiles of 128
NKP = NKT // 2               # 10 key-tile pairs

_PROG = {}


def _build_program(reps=1):
    from contextlib import ExitStack

    import concourse.bacc as bacc
    import concourse.bass as bass
    import concourse.tile as tile
    from concourse import mybir

    f32 = mybir.dt.float32
    bf = mybir.dt.bfloat16
    f16 = mybir.dt.float16
    AF = mybir.ActivationFunctionType

    nc = bacc.Bacc("TRN2", target_bir_lowering=False)
    xt_d = nc.declare_dram_parameter("xt", [D, L_IMG], bf, isOutput=False)
    et_d = nc.declare_dram_parameter("et", [D, L_TXT], bf, isOutput=False)
    wq_d = nc.declare_dram_parameter("wq", [D, HPC * DH], bf, isOutput=False)
    wk_d = nc.declare_dram_parameter("wk", [D, HPC * DH], bf, isOutput=False)
    wv_d = nc.declare_dram_parameter("wv", [D, HPC * DH], bf, isOutput=False)
    wtk_d = nc.declare_dram_parameter("wtk", [D, HPC * DH], bf, isOutput=False)
    wtv_d = nc.declare_dram_parameter("wtv", [D, HPC * DH], bf, isOutput=False)
    wo_d = nc.declare_dram_parameter("wo", [HPC * DH, D], bf, isOutput=False)
    tq_d = nc.declare_dram_parameter("tq", [DH, 2, L_IMG], bf, isOutput=False)
    tk_d = nc.declare_dram_parameter("tk", [DH, 2, L_IMG], bf, isOutput=False)
    gtk_d = nc.declare_dram_parameter("gtk", [DH, 1], f32, isOutput=False)
    nm_d = nc.declare_dram_parameter("nm", [1, 1], f32, isOutput=False)
    out_d = nc.declare_dram_parameter("out", [L_IMG, D], f32, isOutput=True)

    with tile.TileContext(nc) as tc, ExitStack() as ctx:
        # ---- persistent pools (whole kernel) ----
        const = ctx.enter_context(tc.tile_pool(name="const", bufs=1))
        persist = ctx.enter_context(tc.tile_pool(name="persist", bufs=1))
        # PSUM: ps_s 2x[128,1024] (4 banks) + ps_acc 2x[128,512] (2) +
        # ps_av 1x[128,512] (1) + ps_nb 1x[128,512] (1) = 8 banks
        ps_s = ctx.enter_context(
            tc.tile_pool(name="ps_s", bufs=2, space=bass.MemorySpace.PSUM))
        ps_acc = ctx.enter_context(
            tc.tile_pool(name="ps_acc", bufs=2, space=bass.MemorySpace.PSUM))

        ones_col = const.tile([128, 1], bf, name="ones_col")
        nc.vector.memset(ones_col[:], 1.0)
        ones_row_h = const.tile([1, 128], f16, name="ones_row_h")
        nc.vector.memset(ones_row_h[:], 1.0)
        eps_s = const.tile([1, 1], f32, name="eps_s")
        nc.vector.memset(eps_s[:], EPS)
        gtk_s = const.tile([DH, 1], f32, name="gtk_s")
        nm_s = const.tile([1, 1], f32, name="nm_s")
        tqs = const.tile([DH, 2, L_IMG], bf, name="tqs")
        tks_tab = const.tile([DH, 2, L_IMG], bf, name="tks_tab")

        qf = [persist.tile([DH, L_IMG], bf, name=f"qf{h}", tag=f"qf{h}")
              for h in range(HPC)]
        kf = [persist.tile([DH, L_IMG], bf, name=f"kf{h}", tag=f"kf{h}")
              for h in range(HPC)]
        tkf = [persist.tile([DH, L_TXT], bf, name=f"tkf{h}", tag=f"tkf{h}")
               for h in range(HPC)]
        vs = persist.tile([128, NKT, HPC * DH], bf, name="vs")

        def rmsnorm_factor(pool_small, acc_tile, ps_nb):
            """acc_tile: [128, n] f32 projection accumulator tile. Returns a
            [128, n] f32 PSUM broadcast of rsqrt(mean(x^2) + eps) per col."""
            n = acc_tile.shape[-1]
            sqt = pool_small.tile([128, n], bf, name="sqt", tag="sqt", bufs=2)
            nc.scalar.square(sqt[:], acc_tile[:])
            # reduce into row 0 of the (now dead) projection accumulator --
            # avoids burning a PSUM bank and keeps the acc rotation parity
            nc.tensor.matmul(acc_tile[0:1, :], ones_col[:], sqt[:],
                             start=True, stop=True)
            sq = pool_small.tile([1, n], f32, name="sq", tag="sq", bufs=2)
            nc.scalar.activation(sq[:], acc_tile[0:1, :], AF.Sqrt,
                                 bias=eps_s[:], scale=1.0 / DH)
            rnh = pool_small.tile([1, n], f16, name="rnh", tag="rnh", bufs=2)
            with nc.allow_low_precision(reason="f16 norm factor, |x|~1"):
                nc.vector.reciprocal(rnh[:], sq[:])
            nb = ps_nb.tile([128, n], f32, name="nb", tag="nb")
            nc.tensor.matmul(nb[:], ones_row_h[:], rnh[:], start=True,
                             stop=True)
            return nb

        for _ in range(reps):
            with tc.tile_pool(name="phP0", bufs=1) as phP0, \
                 tc.tile_pool(name="ps_nb", bufs=2,
                              space=bass.MemorySpace.PSUM) as ps_nb:
                wqs = phP0.tile([128, NDT, HPC * DH], bf, name="wqs")
                wks = phP0.tile([128, NDT, HPC * DH], bf, name="wks")
                with tc.tile_pool(name="phT", bufs=1) as phT:
                    ets = phT.tile([128, NDT, L_TXT], bf, name="ets")
                    wtks = phT.tile([128, NDT, HPC * DH], bf, name="wtks")
                    wtvs = phT.tile([128, NDT, HPC * DH], bf, name="wtvs")
                    xs0 = phP0.tile([128, NDT, 512], bf, name="xs", tag="xs",
                                    bufs=2)
                    # DMA issue order is queue-execution order. SP gets the
                    # first-needed loads; Act queue takes wtks/tks (idle til
                    # T compute starts).
                    nc.sync.dma_start(gtk_s[:], gtk_d[:, :])
                    nc.sync.dma_start(nm_s[:], nm_d[:, :])
                    et_r = et_d[:, :].rearrange("(t p) l -> p t l", p=128)
                    wtk_r = wtk_d[:, :].rearrange("(t p) m -> p t m", p=128)
                    hd = NDT // 2
                    nc.sync.dma_start(ets[:, 0:hd, :], et_r[:, 0:hd, :])
                    nc.scalar.dma_start(wtks[:, 0:hd, :], wtk_r[:, 0:hd, :])
                    nc.sync.dma_start(ets[:, hd:NDT, :], et_r[:, hd:NDT, :])
                    nc.scalar.dma_start(wtks[:, hd:NDT, :], wtk_r[:, hd:NDT, :])
                    nc.scalar.dma_start(tks_tab[:], tk_d[:, :, :])
                    nc.sync.dma_start(tqs[:], tq_d[:, :, :])
                    nc.sync.dma_start(
                        wtvs[:],
                        wtv_d[:, :].rearrange("(t p) m -> p t m", p=128))
                    nc.sync.dma_start(
                        wqs[:], wq_d[:, :].rearrange("(t p) m -> p t m", p=128))
                    nc.sync.dma_start(
                        wks[:], wk_d[:, :].rearrange("(t p) m -> p t m", p=128))
                    xt_r = xt_d[:, :].rearrange("(t p) l -> p t l", p=128)
                    nc.sync.dma_start(xs0[:], xt_r[:, :, 0:512])

                    # ================= phase T: text k/v =================
                    with tc.tile_pool(name="phTt", bufs=2) as phTt:
                        for h in range(HPC):
                            kp = ps_acc.tile([128, L_TXT], f32, name="kp",
                                             tag="acc")
                            for d in range(NDT):
                                nc.tensor.matmul(
                                    kp[:], wtks[:, d, h * DH:(h + 1) * DH],
                                    ets[:, d, :],
                                    start=(d == 0), stop=(d == NDT - 1))
                            ksc = phTt.tile([128, L_TXT], bf, name="ksc",
                                            tag="ksc")
                            nc.scalar.activation(ksc[:], kp[:], AF.Copy,
                                                 scale=gtk_s[:])
                            nb = rmsnorm_factor(phTt, kp, ps_nb)
                            nc.vector.tensor_mul(tkf[h][:, :], ksc[:], nb[:])
                        for lt in range(NKT_TXT):
                            vp = ps_acc.tile([128, HPC * DH], f32, name="vp",
                                             tag="acc")
                            for d in range(NDT):
                                nc.tensor.matmul(
                                    vp[:], ets[:, d, lt * 128:(lt + 1) * 128],
                                    wtvs[:, d, :],
                                    start=(d == 0), stop=(d == NDT - 1))
                            nc.scalar.copy(vs[:, lt, :], vp[:])

                # ========= phase P: image q/k/v projections =========
                with tc.tile_pool(name="phP1", bufs=1) as phP1, \
                     tc.tile_pool(name="phPt", bufs=2) as phPt:
                    wvs = phP1.tile([128, NDT, HPC * DH], bf, name="wvs")
                    nc.sync.dma_start(
                        wvs[:], wv_d[:, :].rearrange("(t p) m -> p t m", p=128))
                    xs_cur = xs0
                    for lc in range(NLC):
                        lsl = slice(lc * 512, (lc + 1) * 512)
                        xs = xs_cur
                        if lc + 1 < NLC:
                            xs_cur = phP0.tile([128, NDT, 512], bf, name="xs",
                                               tag="xs", bufs=2)
                            nc.sync.dma_start(
                                xs_cur[:],
                                xt_r[:, :, (lc + 1) * 512:(lc + 2) * 512])
                        for h in range(HPC):
                            for wt, tab, dst in ((wqs, tqs, qf[h]),
                                                 (wks, tks_tab, kf[h])):
                                pp = ps_acc.tile([128, 512], f32, name="pp",
                                                 tag="acc")
                                for d in range(NDT):
                                    nc.tensor.matmul(
                                        pp[:], wt[:, d, h * DH:(h + 1) * DH],
                                        xs[:, d, :],
                                        start=(d == 0), stop=(d == NDT - 1))
                                ev = phPt.tile([128, 512], bf, name="ev",
                                               tag="ev")
                                nc.scalar.copy(ev[:], pp[:])
                                nb = rmsnorm_factor(phPt, pp, ps_nb)
                                # rope then norm:
                                # dst = (tabA*ev + tabB*swap64(ev)) * nb
                                evsA = phPt.tile([128, 512], bf, name="evsA",
                                                 tag="evsA")
                                nc.sync.dma_start(evsA[0:64, :], ev[64:128, :])
                                evsB = phPt.tile([128, 512], bf, name="evsB",
                                                 tag="evsB")
                                nc.sync.dma_start(evsB[64:128, :], ev[0:64, :])
                                rA = phPt.tile([128, 512], bf, name="rA",
                                               tag="rA")
                                nc.gpsimd.tensor_mul(rA[:], ev[:],
                                                     tab[:, 0, lsl])
                                rB = phPt.tile([128, 512], bf, name="rB",
                                               tag="rB")
                                nc.vector.tensor_mul(rB[0:64, :], evsA[0:64, :],
                                                     tab[0:64, 1, lsl])
                                nc.vector.tensor_mul(rB[64:128, :],
                                                     evsB[64:128, :],
                                                     tab[64:128, 1, lsl])
                                rs = phPt.tile([128, 512], bf, name="rs",
                                               tag="rs")
                                nc.gpsimd.tensor_add(rs[:], rA[:], rB[:])
                                nc.vector.tensor_mul(dst[:, lsl], rs[:], nb[:])
                        for ltl in range(4):
                            vp = ps_acc.tile([128, HPC * DH], f32, name="vpi",
                                             tag="acc")
                            for d in range(NDT):
                                nc.tensor.matmul(
                                    vp[:], xs[:, d, ltl * 128:(ltl + 1) * 128],
                                    wvs[:, d, :], start=(d == 0),
                                    stop=(d == NDT - 1))
                            nc.scalar.copy(vs[:, NKT_TXT + lc * 4 + ltl, :],
                                           vp[:])

            # ====== phase A+O: attention fused with out-projection ======
            with tc.tile_pool(name="phA", bufs=1) as phA, \
                 tc.tile_pool(name="phAt", bufs=2) as phAt, \
                 tc.tile_pool(name="phAv", bufs=2) as phAv, \
                 tc.tile_pool(name="phOt", bufs=3) as phOt, \
                 tc.tile_pool(name="ps_av", bufs=2,
                              space=bass.MemorySpace.PSUM) as ps_av:
                wos = [phA.tile([DH, D], bf, name=f"wos{h}", tag=f"wos{h}")
                       for h in range(HPC)]
                for h in range(HPC):
                    nc.sync.dma_start(wos[h][:], wo_d[h * DH:(h + 1) * DH, :])
                # prefetch the exp table set while PE starts on scores
                dumm = phAt.tile([1, 1], f32, name="dumm", tag="dumm")
                nc.scalar.activation(dumm[:], eps_s[:], AF.Exp)

                pending_fins = []
                ready_ops = []

                def run_slot(ops_first=False):
                    # ready_ops only holds chunks whose afv is fully written
                    if ops_first and ready_ops:
                        ready_ops.pop(0)()
                    elif pending_fins:
                        pending_fins.pop(0)()
                    elif ready_ops:
                        ready_ops.pop(0)()

                def make_finA(paS, nm_s=nm_s):
                    zs = phAt.tile([1, 512], f32, name="zs", tag="zs")
                    rzh = phAt.tile([1, 512], f16, name="rzh", tag="rzh")

                    def finA():
                        zpt = ps_acc.tile([128, 512], f32, name="zpa",
                                          tag="acc")
                        nc.tensor.matmul(zpt[0:1, :], ones_col[:], paS[:],
                                         start=True, stop=True)
                        nc.scalar.add(zs[:], zpt[0:1, :], nm_s[:])
                        with nc.allow_low_precision(reason="f16 1/Z factor"):
                            nc.vector.reciprocal(rzh[:], zs[:])
                    return finA, rzh

                def make_finB(rzh, avs, afv, h, release=None):
                    def finB():
                        nb2 = ps_acc.tile([128, 512], f32, name="nb2",
                                          tag="acc")
                        nc.tensor.matmul(nb2[:], ones_row_h[:], rzh[:],
                                         start=True, stop=True)
                        nc.vector.tensor_mul(afv[:, h, :], avs[:], nb2[:])
                        if release is not None:
                            ready_ops.extend(release)
                    return finB

                def make_opchunk(afv, lqc, ltl, dc, on_act=False):
                    def opchunk():
                        op = ps_acc.tile([128, 512], f32, name="op", tag="acc")
                        for hh in range(HPC):
                            nc.tensor.matmul(
                                op[:], afv[:, hh, ltl * 128:(ltl + 1) * 128],
                                wos[hh][:, dc * 512:(dc + 1) * 512],
                                start=(hh == 0), stop=(hh == HPC - 1))
                        os_t = phOt.tile([128, 512], f32, name="os", tag="os")
                        nc.vector.tensor_copy(os_t[:], op[:])
                        row0 = lqc * 512 + ltl * 128
                        eng = nc.scalar if on_act else nc.sync
                        eng.dma_start(
                            out_d[row0:row0 + 128, dc * 512:(dc + 1) * 512],
                            os_t[:])
                    return opchunk

                PEND_SLOTS = frozenset((4, 6, 8, 10))
                for lqc in range(NLC):
                    qsl = slice(lqc * 512, (lqc + 1) * 512)
                    afv = phAv.tile([128, HPC, 512], bf, name="afv", tag="afv")
                    rel = []
                    for h in range(HPC):
                        paP = phAt.tile([128, 512], f32, name="paP", tag="paP")
                        paD = phAt.tile([128, 512], f32, name="paD", tag="paD")
                        av = ps_av.tile([128, 512], f32, name="av", tag="av")
                        for lkp in range(NKP + 1):
                            if lkp < NKP:
                                spw = ps_s.tile([128, 1024], f32, name="spw",
                                                tag="s")
                                pte = phA.tile([128, 1024], bf, name="pt",
                                               tag="pt", bufs=2)
                                for half in range(2):
                                    lk = 2 * lkp + half
                                    if lk < NKT_TXT:
                                        lhsT = tkf[h][:, lk * 128:
                                                      (lk + 1) * 128]
                                    else:
                                        lhsT = kf[h][:, (lk - NKT_TXT) * 128:
                                                     (lk - NKT_TXT + 1) * 128]
                                    nc.tensor.matmul(
                                        spw[:, half * 512:(half + 1) * 512],
                                        lhsT, qf[h][:, qsl],
                                        start=True, stop=True)
                                nc.scalar.activation(pte[:], spw[:], AF.Exp,
                                                     scale=SM_SCALE)
                            # pre-AV slots: at the head-loop start the first
                            # AV waits on exp0's Act latency; emit dependency-
                            # free out-proj work BETWEEN S-pair and AV as cover
                            if lkp in (0, 1):
                                run_slot(ops_first=True)
                            jp = lkp - 1
                            if jp >= 0:
                                for half in range(2):
                                    j = 2 * jp + half
                                    pj = ptp[:, half * 512:(half + 1) * 512]
                                    nc.tensor.matmul(
                                        av[:], vs[:, j, h * DH:(h + 1) * DH],
                                        pj, start=(j == 0),
                                        stop=(j == NKT - 1))
                                    if half == 0:
                                        if jp == 0:
                                            nc.gpsimd.tensor_copy(paP[:], pj)
                                        else:
                                            nc.gpsimd.tensor_add(paP[:],
                                                                 paP[:], pj)
                                    else:
                                        if jp == 0:
                                            nc.vector.tensor_copy(paD[:], pj)
                                        else:
                                            nc.vector.tensor_add(paD[:],
                                                                 paD[:], pj)
                            ptp = pte if lkp < NKP else None
                            if lkp in PEND_SLOTS:
                                run_slot()
                        paS = phAt.tile([128, 512], bf, name="paS", tag="paS")
                        nc.vector.tensor_add(paS[:], paP[:], paD[:])
                        avs = phAt.tile([128, 512], f32, name="avs", tag="avs")
                        nc.vector.tensor_copy(avs[:], av[:])
                        finA, rzh = make_finA(paS)
                        pending_fins.append(finA)
                        pending_fins.append(make_finB(
                            rzh, avs, afv, h,
                            release=rel if h == HPC - 1 else None))
                    last = lqc == NLC - 1
                    for ltl in range(4):
                        for dc in range(D // 512):
                            rel.append(make_opchunk(
                                afv, lqc, ltl, dc,
                                on_act=last and (ltl + dc) % 2 == 1))
                while pending_fins or ready_ops:
                    run_slot()

    nc.finalize()
    return nc


def _get_program(reps=1):
    if reps not in _PROG:
        _PROG[reps] = _build_program(reps=reps)
    return _PROG[reps]


_PERM = np.concatenate([np.arange(0, DH, 2), np.arange(1, DH, 2)])


def make_core_inputs(inputs: dict) -> list:
    hs = np.asarray(inputs["hidden_states"], np.float32)
    enc = np.asarray(inputs["encoder_hidden_states"], np.float32)
    mask = np.asarray(inputs["attention_mask"]).astype(bool)
    emb = np.asarray(inputs["image_rotary_emb"], np.float32)
    wqkv = np.asarray(inputs["w_img_qkv"], np.float32).reshape(D, 3, H, DH)
    wtkv = np.asarray(inputs["w_txt_kv"], np.float32).reshape(D, 2, H, DH)
    wout = np.asarray(inputs["w_out"], np.float32).reshape(H, DH, D)
    g_q = np.asarray(inputs["g_q"], np.float32)
    g_k = np.asarray(inputs["g_k"], np.float32)
    g_ak = np.asarray(inputs["g_added_k"], np.float32)

    def tables(F, g):
        # F: [L, 64, 2, 2]; permuted layout: part p<64 -> dim 2p, 64+p -> 2p+1
        # dst = tabA * ev + tabB * swap64(ev)
        ge, go = g[0::2], g[1::2]
        tabA = np.concatenate([(F[:, :, 0, 0] * ge[None, :]).T,
                               (F[:, :, 1, 1] * go[None, :]).T], axis=0)
        tabB = np.concatenate([(F[:, :, 0, 1] * go[None, :]).T,
                               (F[:, :, 1, 0] * ge[None, :]).T], axis=0)
        return np.stack([tabA, tabB], axis=1).astype(bf16)  # [128, 2, L]

    in_maps = []
    for c in range(NCORES):
        b, g = divmod(c, 4)
        hsel = slice(g * HPC, (g + 1) * HPC)
        F = emb[b, 0]
        wq = wqkv[:, 0, hsel, :][:, :, _PERM].reshape(D, HPC * DH)
        wk = wqkv[:, 1, hsel, :][:, :, _PERM].reshape(D, HPC * DH)
        wv = wqkv[:, 2, hsel, :].reshape(D, HPC * DH)
        wtk = wtkv[:, 0, hsel, :][:, :, _PERM].reshape(D, HPC * DH)
        wtv = wtkv[:, 1, hsel, :].reshape(D, HPC * DH)
        wo = wout[hsel].reshape(HPC * DH, D)
        in_maps.append({
            "xt": np.ascontiguousarray(hs[b].T).astype(bf16),
            "et": np.ascontiguousarray((enc[b] * mask[b][:, None]).T).astype(bf16),
            "wq": np.ascontiguousarray(wq).astype(bf16),
            "wk": np.ascontiguousarray(wk).astype(bf16),
            "wv": np.ascontiguousarray(wv).astype(bf16),
            "wtk": np.ascontiguousarray(wtk).astype(bf16),
            "wtv": np.ascontiguousarray(wtv).astype(bf16),
            "wo": np.ascontiguousarray(wo).astype(bf16),
            "tq": tables(F, g_q),
            "tk": tables(F, g_k),
            "gtk": g_ak[_PERM].reshape(DH, 1).astype(np.float32),
            "nm": np.array([[-(float(L_TXT) - float(mask[b].sum()))]], np.float32),
        })
    return in_maps


def run_cores(in_maps, trace=False, tmpdir=None):
    from concourse.bass_utils import run_bass_kernel_spmd
    nc = _get_program()
    return run_bass_kernel_spmd(nc, in_maps, list(range(NCORES)),
                                trace=trace, tmpdir=tmpdir)


def kernel(**inputs) -> np.ndarray:
    in_maps = make_core_inputs(inputs)
    res = run_cores(in_maps)
    out = np.zeros((B, L_IMG, D), np.float32)
    for c in range(NCORES):
        b = c // 4
        out[b] += np.asarray(res.results[c]["out"], np.float32)
    return out


# revision 45
# speedup vs baseline: 1.0378x; 1.0378x over previous
"""PRXAttention TRN2 kernel: 8-core SPMD (2 batches x 4 head-groups).

Per core (b, g): project q/k/v for 4 heads (img) + k/v (txt), RMSNorm via
PE ones-matmul partition reduction, RoPE with host-prepared pair-deinterleaved
tables (g_q/g_k folded in), softmax without max-subtraction (scores bounded),
masking via host-side zeroing of masked encoder tokens + Z correction,
partial out-projection for the 4 heads; host sums the 4 partials per batch.

Engine plan: PE is the pacer everywhere. Input DMAs are split across the SP
and Act HWDGE queues and prefetched a phase early. In attention, the softmax
Z chain and the out-projection matmuls are deferred into a pending-work queue
drained one item per key-loop iteration of the NEXT head, so the PE stream
always has ready work while Act does exp.
"""

import numpy as np
import ml_dtypes

bf16 = ml_dtypes.bfloat16

B, L_IMG, L_TXT = 2, 2048, 2048 // 4
D, H, DH = 2048, 16, 128
HPC = 4                      # heads per core
NCORES = 8
EPS = 1e-6
SM_SCALE = 1.0 / float(np.sqrt(DH))
NDT = D // 128               # 16 d-model tiles
NLC = L_IMG // 512           # 4 img l-chunks of 512
NKT_TXT = L_TXT // 128       # 4 txt key tiles
NKT = NKT_TXT + L_IMG // 128  # 20 key tiles of 128
NKP = NKT // 2               # 10 key-tile pairs

_PROG = {}


def _build_program(reps=1):
    from contextlib import ExitStack

    import concourse.bacc as bacc
    import concourse.bass as bass
    import concourse.tile as tile
    from concourse import mybir

    f32 = mybir.dt.float32
    bf = mybir.dt.bfloat16
    f16 = mybir.dt.float16
    AF = mybir.ActivationFunctionType

    nc = bacc.Bacc("TRN2", target_bir_lowering=False)
    xt_d = nc.declare_dram_parameter("xt", [D, L_IMG], bf, isOutput=False)
    et_d = nc.declare_dram_parameter("et", [D, L_TXT], bf, isOutput=False)
    wq_d = nc.declare_dram_parameter("wq", [D, HPC * DH], bf, isOutput=False)
    wk_d = nc.declare_dram_parameter("wk", [D, HPC * DH], bf, isOutput=False)
    wv_d = nc.declare_dram_parameter("wv", [D, HPC * DH], bf, isOutput=False)
    wtk_d = nc.declare_dram_parameter("wtk", [D, HPC * DH], bf, isOutput=False)
    wtv_d = nc.declare_dram_parameter("wtv", [D, HPC * DH], bf, isOutput=False)
    wo_d = nc.declare_dram_parameter("wo", [HPC * DH, D], bf, isOutput=False)
    tq_d = nc.declare_dram_parameter("tq", [DH, 2, L_IMG], bf, isOutput=False)
    tk_d = nc.declare_dram_parameter("tk", [DH, 2, L_IMG], bf, isOutput=False)
    gtk_d = nc.declare_dram_parameter("gtk", [DH, 1], f32, isOutput=False)
    nm_d = nc.declare_dram_parameter("nm", [1, 1], f32, isOutput=False)
    out_d = nc.declare_dram_parameter("out", [L_IMG, D], f32, isOutput=True)

    with tile.TileContext(nc) as tc, ExitStack() as ctx:
        # ---- persistent pools (whole kernel) ----
        const = ctx.enter_context(tc.tile_pool(name="const", bufs=1))
        persist = ctx.enter_context(tc.tile_pool(name="persist", bufs=1))
        # PSUM: ps_s 2x[128,1024] (4 banks) + ps_acc 2x[128,512] (2) +
        # ps_av 1x[128,512] (1) + ps_nb 1x[128,512] (1) = 8 banks
        ps_s = ctx.enter_context(
            tc.tile_pool(name="ps_s", bufs=2, space=bass.MemorySpace.PSUM))
        ps_acc = ctx.enter_context(
            tc.tile_pool(name="ps_acc", bufs=2, space=bass.MemorySpace.PSUM))

        ones_col = const.tile([128, 1], bf, name="ones_col")
        nc.vector.memset(ones_col[:], 1.0)
        ones_row_h = const.tile([1, 128], f16, name="ones_row_h")
        nc.vector.memset(ones_row_h[:], 1.0)
        eps_s = const.tile([1, 1], f32, name="eps_s")
        nc.vector.memset(eps_s[:], EPS)
        gtk_s = const.tile([DH, 1], f32, name="gtk_s")
        nm_s = const.tile([1, 1], f32, name="nm_s")
        tqs = const.tile([DH, 2, L_IMG], bf, name="tqs")
        tks_tab = const.tile([DH, 2, L_IMG], bf, name="tks_tab")

        qf = [persist.tile([DH, L_IMG], bf, name=f"qf{h}", tag=f"qf{h}")
              for h in range(HPC)]
        kf = [persist.tile([DH, L_IMG], bf, name=f"kf{h}", tag=f"kf{h}")
              for h in range(HPC)]
        tkf = [persist.tile([DH, L_TXT], bf, name=f"tkf{h}", tag=f"tkf{h}")
               for h in range(HPC)]
        vs = persist.tile([128, NKT, HPC * DH], bf, name="vs")

        def rmsnorm_factor(pool_small, acc_tile, ps_nb):
            """acc_tile: [128, n] f32 projection accumulator tile. Returns a
            [128, n] f32 PSUM broadcast of rsqrt(mean(x^2) + eps) per col."""
            n = acc_tile.shape[-1]
            sqt = pool_small.tile([128, n], bf, name="sqt", tag="sqt", bufs=2)
            nc.scalar.square(sqt[:], acc_tile[:])
            # reduce into row 0 of the (now dead) projection accumulator --
            # avoids burning a PSUM bank and keeps the acc rotation parity
            nc.tensor.matmul(acc_tile[0:1, :], ones_col[:], sqt[:],
                             start=True, stop=True)
            sq = pool_small.tile([1, n], f32, name="sq", tag="sq", bufs=2)
            nc.scalar.activation(sq[:], acc_tile[0:1, :], AF.Sqrt,
                                 bias=eps_s[:], scale=1.0 / DH)
            rnh = pool_small.tile([1, n], f16, name="rnh", tag="rnh", bufs=2)
            with nc.allow_low_precision(reason="f16 norm factor, |x|~1"):
                nc.vector.reciprocal(rnh[:], sq[:])
            nb = ps_nb.tile([128, n], f32, name="nb", tag="nb")
            nc.tensor.matmul(nb[:], ones_row_h[:], rnh[:], start=True,
                             stop=True)
            return nb

        for _ in range(reps):
            with tc.tile_pool(name="phP0", bufs=1) as phP0, \
                 tc.tile_pool(name="ps_nb", bufs=2,
                              space=bass.MemorySpace.PSUM) as ps_nb:
                wqs = phP0.tile([128, NDT, HPC * DH], bf, name="wqs")
                wks = phP0.tile([128, NDT, HPC * DH], bf, name="wks")
                with tc.tile_pool(name="phT", bufs=1) as phT:
                    ets = phT.tile([128, NDT, L_TXT], bf, name="ets")
                    wtks = phT.tile([128, NDT, HPC * DH], bf, name="wtks")
                    wtvs = phT.tile([128, NDT, HPC * DH], bf, name="wtvs")
                    xs0 = phP0.tile([128, NDT, 512], bf, name="xs", tag="xs",
                                    bufs=2)
                    # DMA issue order is queue-execution order. SP gets the
                    # first-needed loads; Act queue takes wtks/tks (idle til
                    # T compute starts).
                    nc.sync.dma_start(gtk_s[:], gtk_d[:, :])
                    nc.sync.dma_start(nm_s[:], nm_d[:, :])
                    et_r = et_d[:, :].rearrange("(t p) l -> p t l", p=128)
                    wtk_r = wtk_d[:, :].rearrange("(t p) m -> p t m", p=128)
                    hd = NDT // 2
                    nc.sync.dma_start(ets[:, 0:hd, :], et_r[:, 0:hd, :])
                    nc.scalar.dma_start(wtks[:, 0:hd, :], wtk_r[:, 0:hd, :])
                    nc.sync.dma_start(ets[:, hd:NDT, :], et_r[:, hd:NDT, :])
                    nc.scalar.dma_start(wtks[:, hd:NDT, :], wtk_r[:, hd:NDT, :])
                    nc.scalar.dma_start(tks_tab[:], tk_d[:, :, :])
                    nc.sync.dma_start(tqs[:], tq_d[:, :, :])
                    nc.sync.dma_start(
                        wtvs[:],
                        wtv_d[:, :].rearrange("(t p) m -> p t m", p=128))
                    nc.sync.dma_start(
                        wqs[:], wq_d[:, :].rearrange("(t p) m -> p t m", p=128))
                    nc.sync.dma_start(
                        wks[:], wk_d[:, :].rearrange("(t p) m -> p t m", p=128))
                    xt_r = xt_d[:, :].rearrange("(t p) l -> p t l", p=128)
                    nc.sync.dma_start(xs0[:], xt_r[:, :, 0:512])

                    # ================= phase T: text k/v =================
                    with tc.tile_pool(name="phTt", bufs=2) as phTt:
                        for h in range(HPC):
                            kp = ps_acc.tile([128, L_TXT], f32, name="kp",
                                             tag="acc")
                            for d in range(NDT):
                                nc.tensor.matmul(
                                    kp[:], wtks[:, d, h * DH:(h + 1) * DH],
                                    ets[:, d, :],
                                    start=(d == 0), stop=(d == NDT - 1))
                            ksc = phTt.tile([128, L_TXT], bf, name="ksc",
                                            tag="ksc")
                            nc.scalar.activation(ksc[:], kp[:], AF.Copy,
                                                 scale=gtk_s[:])
                            nb = rmsnorm_factor(phTt, kp, ps_nb)
                            nc.vector.tensor_mul(tkf[h][:, :], ksc[:], nb[:])
                        for lt in range(NKT_TXT):
                            vp = ps_acc.tile([128, HPC * DH], f32, name="vp",
                                             tag="acc")
                            for d in range(NDT):
                                nc.tensor.matmul(
                                    vp[:], ets[:, d, lt * 128:(lt + 1) * 128],
                                    wtvs[:, d, :],
                                    start=(d == 0), stop=(d == NDT - 1))
                            nc.scalar.copy(vs[:, lt, :], vp[:])

                # ========= phase P: image q/k/v projections =========
                with tc.tile_pool(name="phP1", bufs=1) as phP1, \
                     tc.tile_pool(name="phPt", bufs=2) as phPt:
                    wvs = phP1.tile([128, NDT, HPC * DH], bf, name="wvs")
                    nc.sync.dma_start(
                        wvs[:], wv_d[:, :].rearrange("(t p) m -> p t m", p=128))
                    xs_cur = xs0
                    for lc in range(NLC):
                        lsl = slice(lc * 512, (lc + 1) * 512)
                        xs = xs_cur
                        if lc + 1 < NLC:
                            xs_cur = phP0.tile([128, NDT, 512], bf, name="xs",
                                               tag="xs", bufs=2)
                            nc.sync.dma_start(
                                xs_cur[:],
                                xt_r[:, :, (lc + 1) * 512:(lc + 2) * 512])
                        for h in range(HPC):
                            for wt, tab, dst in ((wqs, tqs, qf[h]),
                                                 (wks, tks_tab, kf[h])):
                                pp = ps_acc.tile([128, 512], f32, name="pp",
                                                 tag="acc")
                                for d in range(NDT):
                                    nc.tensor.matmul(
                                        pp[:], wt[:, d, h * DH:(h + 1) * DH],
                                        xs[:, d, :],
                                        start=(d == 0), stop=(d == NDT - 1))
                                ev = phPt.tile([128, 512], bf, name="ev",
                                               tag="ev")
                                nc.scalar.copy(ev[:], pp[:])
                                nb = rmsnorm_factor(phPt, pp, ps_nb)
                                # rope then norm:
                                # dst = (tabA*ev + tabB*swap64(ev)) * nb
                                evsA = phPt.tile([128, 512], bf, name="evsA",
                                                 tag="evsA")
                                nc.sync.dma_start(evsA[0:64, :], ev[64:128, :])
                                evsB = phPt.tile([128, 512], bf, name="evsB",
                                                 tag="evsB")
                                nc.sync.dma_start(evsB[64:128, :], ev[0:64, :])
                                rA = phPt.tile([128, 512], bf, name="rA",
                                               tag="rA")
                                nc.gpsimd.tensor_mul(rA[:], ev[:],
                                                     tab[:, 0, lsl])
                                rB = phPt.tile([128, 512], bf, name="rB",
                                               tag="rB")
                                nc.vector.tensor_mul(rB[0:64, :], evsA[0:64, :],
                                                     tab[0:64, 1, lsl])
                                nc.vector.tensor_mul(rB[64:128, :],
                                                     evsB[64:128, :],
                                                     tab[64:128, 1, lsl])
                                rs = phPt.tile([128, 512], bf, name="rs",
                                               tag="rs")
                                nc.gpsimd.tensor_add(rs[:], rA[:], rB[:])
                                nc.vector.tensor_mul(dst[:, lsl], rs[:], nb[:])
                        for ltl in range(4):
                            vp = ps_acc.tile([128, HPC * DH], f32, name="vpi",
                                             tag="acc")
                            for d in range(NDT):
                                nc.tensor.matmul(
                                    vp[:], xs[:, d, ltl * 128:(ltl + 1) * 128],
                                    wvs[:, d, :], start=(d == 0),
                                    stop=(d == NDT - 1))
                            nc.scalar.copy(vs[:, NKT_TXT + lc * 4 + ltl, :],
                                           vp[:])

            # ====== phase A+O: attention fused with out-projection ======
            with tc.tile_pool(name="phA", bufs=1) as phA, \
                 tc.tile_pool(name="phAt", bufs=2) as phAt, \
                 tc.tile_pool(name="phAv", bufs=2) as phAv, \
                 tc.tile_pool(name="phOt", bufs=3) as phOt, \
                 tc.tile_pool(name="ps_av", bufs=2,
                              space=bass.MemorySpace.PSUM) as ps_av:
                wos = [phA.tile([DH, D], bf, name=f"wos{h}", tag=f"wos{h}")
                       for h in range(HPC)]
                for h in range(HPC):
                    nc.sync.dma_start(wos[h][:], wo_d[h * DH:(h + 1) * DH, :])
                # prefetch the exp table set while PE starts on scores
                dumm = phAt.tile([1, 1], f32, name="dumm", tag="dumm")
                nc.scalar.activation(dumm[:], eps_s[:], AF.Exp)

                pending_fins = []
                pending_ops = []

                def run_slot():
                    if pending_fins:
                        pending_fins.pop(0)()
                    elif pending_ops:
                        pending_ops.pop(0)()

                def make_finA(paS, nm_s=nm_s):
                    zs = phAt.tile([1, 512], f32, name="zs", tag="zs")
                    rzh = phAt.tile([1, 512], f16, name="rzh", tag="rzh")

                    def finA():
                        zpt = ps_acc.tile([128, 512], f32, name="zpa",
                                          tag="acc")
                        nc.tensor.matmul(zpt[0:1, :], ones_col[:], paS[:],
                                         start=True, stop=True)
                        nc.scalar.add(zs[:], zpt[0:1, :], nm_s[:])
                        with nc.allow_low_precision(reason="f16 1/Z factor"):
                            nc.vector.reciprocal(rzh[:], zs[:])
                    return finA, rzh

                def make_finB(rzh, avs, afv, h):
                    def finB():
                        nb2 = ps_acc.tile([128, 512], f32, name="nb2",
                                          tag="acc")
                        nc.tensor.matmul(nb2[:], ones_row_h[:], rzh[:],
                                         start=True, stop=True)
                        nc.vector.tensor_mul(afv[:, h, :], avs[:], nb2[:])
                    return finB

                def make_opchunk(afv, lqc, ltl, dc, on_act=False):
                    def opchunk():
                        op = ps_acc.tile([128, 512], f32, name="op", tag="acc")
                        for hh in range(HPC):
                            nc.tensor.matmul(
                                op[:], afv[:, hh, ltl * 128:(ltl + 1) * 128],
                                wos[hh][:, dc * 512:(dc + 1) * 512],
                                start=(hh == 0), stop=(hh == HPC - 1))
                        os_t = phOt.tile([128, 512], f32, name="os", tag="os")
                        nc.vector.tensor_copy(os_t[:], op[:])
                        row0 = lqc * 512 + ltl * 128
                        eng = nc.scalar if on_act else nc.sync
                        eng.dma_start(
                            out_d[row0:row0 + 128, dc * 512:(dc + 1) * 512],
                            os_t[:])
                    return opchunk

                PEND_SLOTS = frozenset((2, 4, 6, 8, 9, 10))
                for lqc in range(NLC):
                    qsl = slice(lqc * 512, (lqc + 1) * 512)
                    afv = phAv.tile([128, HPC, 512], bf, name="afv", tag="afv")
                    for h in range(HPC):
                        paP = phAt.tile([128, 512], f32, name="paP", tag="paP")
                        paD = phAt.tile([128, 512], f32, name="paD", tag="paD")
                        av = ps_av.tile([128, 512], f32, name="av", tag="av")
                        for lkp in range(NKP + 1):
                            if lkp < NKP:
                                spw = ps_s.tile([128, 1024], f32, name="spw",
                                                tag="s")
                                pte = phA.tile([128, 1024], bf, name="pt",
                                               tag="pt", bufs=2)
                                for half in range(2):
                                    lk = 2 * lkp + half
                                    if lk < NKT_TXT:
                                        lhsT = tkf[h][:, lk * 128:
                                                      (lk + 1) * 128]
                                    else:
                                        lhsT = kf[h][:, (lk - NKT_TXT) * 128:
                                                     (lk - NKT_TXT + 1) * 128]
                                    nc.tensor.matmul(
                                        spw[:, half * 512:(half + 1) * 512],
                                        lhsT, qf[h][:, qsl],
                                        start=True, stop=True)
                                nc.scalar.activation(pte[:], spw[:], AF.Exp,
                                                     scale=SM_SCALE)
                            jp = lkp - 1
                            if jp >= 0:
                                for half in range(2):
                                    j = 2 * jp + half
                                    pj = ptp[:, half * 512:(half + 1) * 512]
                                    nc.tensor.matmul(
                                        av[:], vs[:, j, h * DH:(h + 1) * DH],
                                        pj, start=(j == 0),
                                        stop=(j == NKT - 1))
                                    if half == 0:
                                        if jp == 0:
                                            nc.gpsimd.tensor_copy(paP[:], pj)
                                        else:
                                            nc.gpsimd.tensor_add(paP[:],
                                                                 paP[:], pj)
                                    else:
                                        if jp == 0:
                                            nc.vector.tensor_copy(paD[:], pj)
                                        else:
                                            nc.vector.tensor_add(paD[:],
                                                                 paD[:], pj)
                            ptp = pte if lkp < NKP else None
                            if lkp in PEND_SLOTS:
                                run_slot()
                        paS = phAt.tile([128, 512], bf, name="paS", tag="paS")
                        nc.vector.tensor_add(paS[:], paP[:], paD[:])
                        avs = phAt.tile([128, 512], f32, name="avs", tag="avs")
                        nc.scalar.copy(avs[:], av[:])
                        finA, rzh = make_finA(paS)
                        pending_fins.append(finA)
                        pending_fins.append(make_finB(rzh, avs, afv, h))
                    last = lqc == NLC - 1
                    for ltl in range(4):
                        for dc in range(D // 512):
                            pending_ops.append(make_opchunk(
                                afv, lqc, ltl, dc,
                                on_act=last and (ltl + dc) % 2 == 1))
                while pending_fins or pending_ops:
                    run_slot()

    nc.finalize()
    return nc


def _get_program(reps=1):
    if reps not in _PROG:
        _PROG[reps] = _build_program(reps=reps)
    return _PROG[reps]


_PERM = np.concatenate([np.arange(0, DH, 2), np.arange(1, DH, 2)])


def make_core_inputs(inputs: dict) -> list:
    hs = np.asarray(inputs["hidden_states"], np.float32)
    enc = np.asarray(inputs["encoder_hidden_states"], np.float32)
    mask = np.asarray(inputs["attention_mask"]).astype(bool)
    emb = np.asarray(inputs["image_rotary_emb"], np.float32)
    wqkv = np.asarray(inputs["w_img_qkv"], np.float32).reshape(D, 3, H, DH)
    wtkv = np.asarray(inputs["w_txt_kv"], np.float32).reshape(D, 2, H, DH)
    wout = np.asarray(inputs["w_out"], np.float32).reshape(H, DH, D)
    g_q = np.asarray(inputs["g_q"], np.float32)
    g_k = np.asarray(inputs["g_k"], np.float32)
    g_ak = np.asarray(inputs["g_added_k"], np.float32)

    def tables(F, g):
        # F: [L, 64, 2, 2]; permuted layout: part p<64 -> dim 2p, 64+p -> 2p+1
        # dst = tabA * ev + tabB * swap64(ev)
        ge, go = g[0::2], g[1::2]
        tabA = np.concatenate([(F[:, :, 0, 0] * ge[None, :]).T,
                               (F[:, :, 1, 1] * go[None, :]).T], axis=0)
        tabB = np.concatenate([(F[:, :, 0, 1] * go[None, :]).T,
                               (F[:, :, 1, 0] * ge[None, :]).T], axis=0)
        return np.stack([tabA, tabB], axis=1).astype(bf16)  # [128, 2, L]

    in_maps = []
    for c in range(NCORES):
        b, g = divmod(c, 4)
        hsel = slice(g * HPC, (g + 1) * HPC)
        F = emb[b, 0]
        wq = wqkv[:, 0, hsel, :][:, :, _PERM].reshape(D, HPC * DH)
        wk = wqkv[:, 1, hsel, :][:, :, _PERM].reshape(D, HPC * DH)
        wv = wqkv[:, 2, hsel, :].reshape(D, HPC * DH)
        wtk = wtkv[:, 0, hsel, :][:, :, _PERM].reshape(D, HPC * DH)
        wtv = wtkv[:, 1, hsel, :].reshape(D, HPC * DH)
        wo = wout[hsel].reshape(HPC * DH, D)
        in_maps.append({
            "xt": np.ascontiguousarray(hs[b].T).astype(bf16),
            "et": np.ascontiguousarray((enc[b] * mask[b][:, None]).T).astype(bf16),
            "wq": np.ascontiguousarray(wq).astype(bf16),
            "wk": np.ascontiguousarray(wk).astype(bf16),
            "wv": np.ascontiguousarray(wv).astype(bf16),
            "wtk": np.ascontiguousarray(wtk).astype(bf16),
            "wtv": np.ascontiguousarray(wtv).astype(bf16),
            "wo": np.ascontiguousarray(wo).astype(bf16),
            "tq": tables(F, g_q),
            "tk": tables(F, g_k),
            "gtk": g_ak[_PERM].reshape(DH, 1).astype(np.float32),
            "nm": np.array([[-(float(L_TXT) - float(mask[b].sum()))]], np.float32),
        })
    return in_maps


def run_cores(in_maps, trace=False, tmpdir=None):
    from concourse.bass_utils import run_bass_kernel_spmd
    nc = _get_program()
    return run_bass_kernel_spmd(nc, in_maps, list(range(NCORES)),
                                trace=trace, tmpdir=tmpdir)


def kernel(**inputs) -> np.ndarray:
    in_maps = make_core_inputs(inputs)
    res = run_cores(in_maps)
    out = np.zeros((B, L_IMG, D), np.float32)
    for c in range(NCORES):
        b = c // 4
        out[b] += np.asarray(res.results[c]["out"], np.float32)
    return out


# revision 48
# speedup vs baseline: 1.0438x; 1.0057x over previous
"""PRXAttention TRN2 kernel: 8-core SPMD (2 batches x 4 head-groups).

Per core (b, g): project q/k/v for 4 heads (img) + k/v (txt), RMSNorm via
PE ones-matmul partition reduction, RoPE with host-prepared pair-deinterleaved
tables (g_q/g_k folded in), softmax without max-subtraction (scores bounded),
masking via host-side zeroing of masked encoder tokens + Z correction,
partial out-projection for the 4 heads; host sums the 4 partials per batch.

Engine plan: PE is the pacer everywhere. Input DMAs are split across the SP
and Act HWDGE queues and prefetched a phase early. In attention, the softmax
Z chain and the out-projection matmuls are deferred into a pending-work queue
drained one item per key-loop iteration of the NEXT head, so the PE stream
always has ready work while Act does exp.
"""

import numpy as np
import ml_dtypes

bf16 = ml_dtypes.bfloat16

B, L_IMG, L_TXT = 2, 2048, 2048 // 4
D, H, DH = 2048, 16, 128
HPC = 4                      # heads per core
NCORES = 8
EPS = 1e-6
SM_SCALE = 1.0 / float(np.sqrt(DH))
NDT = D // 128               # 16 d-model tiles
NLC = L_IMG // 512           # 4 img l-chunks of 512
NKT_TXT = L_TXT // 128       # 4 txt key tiles
NKT = NKT_TXT + L_IMG // 128  # 20 key tiles of 128
NKP = NKT // 2               # 10 key-tile pairs

_PROG = {}


def _build_program(reps=1):
    from contextlib import ExitStack

    import concourse.bacc as bacc
    import concourse.bass as bass
    import concourse.tile as tile
    from concourse import mybir

    f32 = mybir.dt.float32
    bf = mybir.dt.bfloat16
    f16 = mybir.dt.float16
    AF = mybir.ActivationFunctionType

    nc = bacc.Bacc("TRN2", target_bir_lowering=False)
    xt_d = nc.declare_dram_parameter("xt", [D, L_IMG], bf, isOutput=False)
    et_d = nc.declare_dram_parameter("et", [D, L_TXT], bf, isOutput=False)
    wq_d = nc.declare_dram_parameter("wq", [D, HPC * DH], bf, isOutput=False)
    wk_d = nc.declare_dram_parameter("wk", [D, HPC * DH], bf, isOutput=False)
    wv_d = nc.declare_dram_parameter("wv", [D, HPC * DH], bf, isOutput=False)
    wtk_d = nc.declare_dram_parameter("wtk", [D, HPC * DH], bf, isOutput=False)
    wtv_d = nc.declare_dram_parameter("wtv", [D, HPC * DH], bf, isOutput=False)
    wo_d = nc.declare_dram_parameter("wo", [HPC * DH, D], bf, isOutput=False)
    tq_d = nc.declare_dram_parameter("tq", [DH, 2, L_IMG], bf, isOutput=False)
    tk_d = nc.declare_dram_parameter("tk", [DH, 2, L_IMG], bf, isOutput=False)
    gtk_d = nc.declare_dram_parameter("gtk", [DH, 1], f32, isOutput=False)
    nm_d = nc.declare_dram_parameter("nm", [1, 1], f32, isOutput=False)
    out_d = nc.declare_dram_parameter("out", [L_IMG, D], f32, isOutput=True)

    with tile.TileContext(nc) as tc, ExitStack() as ctx:
        # ---- persistent pools (whole kernel) ----
        const = ctx.enter_context(tc.tile_pool(name="const", bufs=1))
        persist = ctx.enter_context(tc.tile_pool(name="persist", bufs=1))
        # PSUM: ps_s 2x[128,1024] (4 banks) + ps_acc 2x[128,512] (2) +
        # ps_av 1x[128,512] (1) + ps_nb 1x[128,512] (1) = 8 banks
        ps_s = ctx.enter_context(
            tc.tile_pool(name="ps_s", bufs=2, space=bass.MemorySpace.PSUM))
        ps_acc = ctx.enter_context(
            tc.tile_pool(name="ps_acc", bufs=2, space=bass.MemorySpace.PSUM))

        ones_col = const.tile([128, 1], bf, name="ones_col")
        nc.vector.memset(ones_col[:], 1.0)
        ones_row_h = const.tile([1, 128], f16, name="ones_row_h")
        nc.vector.memset(ones_row_h[:], 1.0)
        eps_s = const.tile([1, 1], f32, name="eps_s")
        nc.vector.memset(eps_s[:], EPS)
        gtk_s = const.tile([DH, 1], f32, name="gtk_s")
        nm_s = const.tile([1, 1], f32, name="nm_s")
        tqs = const.tile([DH, 2, L_IMG], bf, name="tqs")
        tks_tab = const.tile([DH, 2, L_IMG], bf, name="tks_tab")

        qf = [persist.tile([DH, L_IMG], bf, name=f"qf{h}", tag=f"qf{h}")
              for h in range(HPC)]
        kf = [persist.tile([DH, L_IMG], bf, name=f"kf{h}", tag=f"kf{h}")
              for h in range(HPC)]
        tkf = [persist.tile([DH, L_TXT], bf, name=f"tkf{h}", tag=f"tkf{h}")
               for h in range(HPC)]
        vs = persist.tile([128, NKT, HPC * DH], bf, name="vs")

        def rmsnorm_factor(pool_small, acc_tile, ps_nb):
            """acc_tile: [128, n] f32 projection accumulator tile. Returns a
            [128, n] f32 PSUM broadcast of rsqrt(mean(x^2) + eps) per col."""
            n = acc_tile.shape[-1]
            sqt = pool_small.tile([128, n], bf, name="sqt", tag="sqt", bufs=2)
            nc.scalar.square(sqt[:], acc_tile[:])
            # reduce into row 0 of the (now dead) projection accumulator --
            # avoids burning a PSUM bank and keeps the acc rotation parity
            nc.tensor.matmul(acc_tile[0:1, :], ones_col[:], sqt[:],
                             start=True, stop=True)
            sq = pool_small.tile([1, n], f32, name="sq", tag="sq", bufs=2)
            nc.scalar.activation(sq[:], acc_tile[0:1, :], AF.Sqrt,
                                 bias=eps_s[:], scale=1.0 / DH)
            rnh = pool_small.tile([1, n], f16, name="rnh", tag="rnh", bufs=2)
            with nc.allow_low_precision(reason="f16 norm factor, |x|~1"):
                nc.vector.reciprocal(rnh[:], sq[:])
            nb = ps_nb.tile([128, n], f32, name="nb", tag="nb")
            nc.tensor.matmul(nb[:], ones_row_h[:], rnh[:], start=True,
                             stop=True)
            return nb

        for _ in range(reps):
            with tc.tile_pool(name="phP0", bufs=1) as phP0, \
                 tc.tile_pool(name="ps_nb", bufs=2,
                              space=bass.MemorySpace.PSUM) as ps_nb:
                wqs = phP0.tile([128, NDT, HPC * DH], bf, name="wqs")
                wks = phP0.tile([128, NDT, HPC * DH], bf, name="wks")
                with tc.tile_pool(name="phT", bufs=1) as phT:
                    ets = phT.tile([128, NDT, L_TXT], bf, name="ets")
                    wtks = phT.tile([128, NDT, HPC * DH], bf, name="wtks")
                    wtvs = phT.tile([128, NDT, HPC * DH], bf, name="wtvs")
                    xs0 = phP0.tile([128, NDT, 512], bf, name="xs", tag="xs",
                                    bufs=2)
                    # DMA issue order is queue-execution order. SP gets the
                    # first-needed loads; Act queue takes wtks/tks (idle til
                    # T compute starts).
                    nc.sync.dma_start(gtk_s[:], gtk_d[:, :])
                    nc.sync.dma_start(nm_s[:], nm_d[:, :])
                    et_r = et_d[:, :].rearrange("(t p) l -> p t l", p=128)
                    wtk_r = wtk_d[:, :].rearrange("(t p) m -> p t m", p=128)
                    hd = NDT // 2
                    nc.sync.dma_start(ets[:, 0:hd, :], et_r[:, 0:hd, :])
                    nc.scalar.dma_start(wtks[:, 0:hd, :], wtk_r[:, 0:hd, :])
                    nc.sync.dma_start(ets[:, hd:NDT, :], et_r[:, hd:NDT, :])
                    nc.scalar.dma_start(wtks[:, hd:NDT, :], wtk_r[:, hd:NDT, :])
                    nc.scalar.dma_start(tks_tab[:], tk_d[:, :, :])
                    nc.sync.dma_start(tqs[:], tq_d[:, :, :])
                    nc.sync.dma_start(
                        wtvs[:],
                        wtv_d[:, :].rearrange("(t p) m -> p t m", p=128))
                    nc.sync.dma_start(
                        wqs[:], wq_d[:, :].rearrange("(t p) m -> p t m", p=128))
                    nc.sync.dma_start(
                        wks[:], wk_d[:, :].rearrange("(t p) m -> p t m", p=128))
                    xt_r = xt_d[:, :].rearrange("(t p) l -> p t l", p=128)
                    nc.sync.dma_start(xs0[:], xt_r[:, :, 0:512])

                    # ================= phase T: text k/v =================
                    with tc.tile_pool(name="phTt", bufs=2) as phTt:
                        for h in range(HPC):
                            kp = ps_acc.tile([128, L_TXT], f32, name="kp",
                                             tag="acc")
                            for d in range(NDT):
                                nc.tensor.matmul(
                                    kp[:], wtks[:, d, h * DH:(h + 1) * DH],
                                    ets[:, d, :],
                                    start=(d == 0), stop=(d == NDT - 1))
                            ksc = phTt.tile([128, L_TXT], bf, name="ksc",
                                            tag="ksc")
                            nc.scalar.activation(ksc[:], kp[:], AF.Copy,
                                                 scale=gtk_s[:])
                            nb = rmsnorm_factor(phTt, kp, ps_nb)
                            nc.vector.tensor_mul(tkf[h][:, :], ksc[:], nb[:])
                        for lt in range(NKT_TXT):
                            vp = ps_acc.tile([128, HPC * DH], f32, name="vp",
                                             tag="acc")
                            for d in range(NDT):
                                nc.tensor.matmul(
                                    vp[:], ets[:, d, lt * 128:(lt + 1) * 128],
                                    wtvs[:, d, :],
                                    start=(d == 0), stop=(d == NDT - 1))
                            nc.scalar.copy(vs[:, lt, :], vp[:])

                # ========= phase P: image q/k/v projections =========
                with tc.tile_pool(name="phP1", bufs=1) as phP1, \
                     tc.tile_pool(name="phPt", bufs=2) as phPt:
                    wvs = phP1.tile([128, NDT, HPC * DH], bf, name="wvs")
                    nc.sync.dma_start(
                        wvs[:], wv_d[:, :].rearrange("(t p) m -> p t m", p=128))
                    xs_cur = xs0
                    for lc in range(NLC):
                        lsl = slice(lc * 512, (lc + 1) * 512)
                        xs = xs_cur
                        if lc + 1 < NLC:
                            xs_cur = phP0.tile([128, NDT, 512], bf, name="xs",
                                               tag="xs", bufs=2)
                            nc.sync.dma_start(
                                xs_cur[:],
                                xt_r[:, :, (lc + 1) * 512:(lc + 2) * 512])
                        for h in range(HPC):
                            for wt, tab, dst in ((wqs, tqs, qf[h]),
                                                 (wks, tks_tab, kf[h])):
                                pp = ps_acc.tile([128, 512], f32, name="pp",
                                                 tag="acc")
                                for d in range(NDT):
                                    nc.tensor.matmul(
                                        pp[:], wt[:, d, h * DH:(h + 1) * DH],
                                        xs[:, d, :],
                                        start=(d == 0), stop=(d == NDT - 1))
                                ev = phPt.tile([128, 512], bf, name="ev",
                                               tag="ev")
                                nc.scalar.copy(ev[:], pp[:])
                                nb = rmsnorm_factor(phPt, pp, ps_nb)
                                # rope then norm:
                                # dst = (tabA*ev + tabB*swap64(ev)) * nb
                                evsA = phPt.tile([128, 512], bf, name="evsA",
                                                 tag="evsA")
                                nc.sync.dma_start(evsA[0:64, :], ev[64:128, :])
                                evsB = phPt.tile([128, 512], bf, name="evsB",
                                                 tag="evsB")
                                nc.sync.dma_start(evsB[64:128, :], ev[0:64, :])
                                rA = phPt.tile([128, 512], bf, name="rA",
                                               tag="rA")
                                nc.gpsimd.tensor_mul(rA[:], ev[:],
                                                     tab[:, 0, lsl])
                                rB = phPt.tile([128, 512], bf, name="rB",
                                               tag="rB")
                                nc.vector.tensor_mul(rB[0:64, :], evsA[0:64, :],
                                                     tab[0:64, 1, lsl])
                                nc.vector.tensor_mul(rB[64:128, :],
                                                     evsB[64:128, :],
                                                     tab[64:128, 1, lsl])
                                rs = phPt.tile([128, 512], bf, name="rs",
                                               tag="rs")
                                nc.gpsimd.tensor_add(rs[:], rA[:], rB[:])
                                nc.vector.tensor_mul(dst[:, lsl], rs[:], nb[:])
                        for ltl in range(4):
                            vp = ps_acc.tile([128, HPC * DH], f32, name="vpi",
                                             tag="acc")
                            for d in range(NDT):
                                nc.tensor.matmul(
                                    vp[:], xs[:, d, ltl * 128:(ltl + 1) * 128],
                                    wvs[:, d, :], start=(d == 0),
                                    stop=(d == NDT - 1))
                            nc.scalar.copy(vs[:, NKT_TXT + lc * 4 + ltl, :],
                                           vp[:])

            # ====== phase A+O: attention fused with out-projection ======
            with tc.tile_pool(name="phA", bufs=1) as phA, \
                 tc.tile_pool(name="phAt", bufs=2) as phAt, \
                 tc.tile_pool(name="phAv", bufs=2) as phAv, \
                 tc.tile_pool(name="phOt", bufs=3) as phOt, \
                 tc.tile_pool(name="ps_av", bufs=2,
                              space=bass.MemorySpace.PSUM) as ps_av:
                wos = [phA.tile([DH, D], bf, name=f"wos{h}", tag=f"wos{h}")
                       for h in range(HPC)]
                for h in range(HPC):
                    nc.sync.dma_start(wos[h][:], wo_d[h * DH:(h + 1) * DH, :])
                # prefetch the exp table set while PE starts on scores
                dumm = phAt.tile([1, 1], f32, name="dumm", tag="dumm")
                nc.scalar.activation(dumm[:], eps_s[:], AF.Exp)

                pending_fins = []
                pending_ops = []

                def run_slot():
                    if pending_fins:
                        pending_fins.pop(0)()
                    elif pending_ops:
                        pending_ops.pop(0)()

                def make_finA(paS, nm_s=nm_s):
                    zs = phAt.tile([1, 512], f32, name="zs", tag="zs")
                    rzh = phAt.tile([1, 512], f16, name="rzh", tag="rzh")

                    def finA():
                        zpt = ps_acc.tile([128, 512], f32, name="zpa",
                                          tag="acc")
                        nc.tensor.matmul(zpt[0:1, :], ones_col[:], paS[:],
                                         start=True, stop=True)
                        nc.scalar.add(zs[:], zpt[0:1, :], nm_s[:])
                        with nc.allow_low_precision(reason="f16 1/Z factor"):
                            nc.vector.reciprocal(rzh[:], zs[:])
                    return finA, rzh

                def make_finB(rzh, avs, afv, h):
                    def finB():
                        nb2 = ps_acc.tile([128, 512], f32, name="nb2",
                                          tag="acc")
                        nc.tensor.matmul(nb2[:], ones_row_h[:], rzh[:],
                                         start=True, stop=True)
                        nc.vector.tensor_mul(afv[:, h, :], avs[:], nb2[:])
                    return finB

                def make_opchunk(afv, lqc, ltl, dc, on_act=False):
                    def opchunk():
                        op = ps_acc.tile([128, 512], f32, name="op", tag="acc")
                        for hh in range(HPC):
                            nc.tensor.matmul(
                                op[:], afv[:, hh, ltl * 128:(ltl + 1) * 128],
                                wos[hh][:, dc * 512:(dc + 1) * 512],
                                start=(hh == 0), stop=(hh == HPC - 1))
                        os_t = phOt.tile([128, 512], f32, name="os", tag="os")
                        nc.vector.tensor_copy(os_t[:], op[:])
                        row0 = lqc * 512 + ltl * 128
                        eng = nc.scalar if on_act else nc.sync
                        eng.dma_start(
                            out_d[row0:row0 + 128, dc * 512:(dc + 1) * 512],
                            os_t[:])
                    return opchunk

                PEND_SLOTS = frozenset((2, 4, 6, 8, 9, 10))
                for lqc in range(NLC):
                    qsl = slice(lqc * 512, (lqc + 1) * 512)
                    afv = phAv.tile([128, HPC, 512], bf, name="afv", tag="afv")
                    for h in range(HPC):
                        paP = phAt.tile([128, 512], f32, name="paP", tag="paP")
                        paD = phAt.tile([128, 512], f32, name="paD", tag="paD")
                        av = ps_av.tile([128, 512], f32, name="av", tag="av")
                        for lkp in range(NKP + 1):
                            if lkp < NKP:
                                spw = ps_s.tile([128, 1024], f32, name="spw",
                                                tag="s")
                                pte = phA.tile([128, 1024], bf, name="pt",
                                               tag="pt", bufs=2)
                                for half in range(2):
                                    lk = 2 * lkp + half
                                    if lk < NKT_TXT:
                                        lhsT = tkf[h][:, lk * 128:
                                                      (lk + 1) * 128]
                                    else:
                                        lhsT = kf[h][:, (lk - NKT_TXT) * 128:
                                                     (lk - NKT_TXT + 1) * 128]
                                    nc.tensor.matmul(
                                        spw[:, half * 512:(half + 1) * 512],
                                        lhsT, qf[h][:, qsl],
                                        start=True, stop=True)
                                nc.scalar.activation(pte[:], spw[:], AF.Exp,
                                                     scale=SM_SCALE)
                            jp = lkp - 1
                            if jp >= 0:
                                for half in range(2):
                                    j = 2 * jp + half
                                    pj = ptp[:, half * 512:(half + 1) * 512]
                                    nc.tensor.matmul(
                                        av[:], vs[:, j, h * DH:(h + 1) * DH],
                                        pj, start=(j == 0),
                                        stop=(j == NKT - 1))
                                    if half == 0:
                                        if jp == 0:
                                            nc.gpsimd.tensor_copy(paP[:], pj)
                                        else:
                                            nc.gpsimd.tensor_add(paP[:],
                                                                 paP[:], pj)
                                    else:
                                        if jp == 0:
                                            nc.vector.tensor_copy(paD[:], pj)
                                        else:
                                            nc.vector.tensor_add(paD[:],
                                                                 paD[:], pj)
                            ptp = pte if lkp < NKP else None
                            if lkp in PEND_SLOTS:
                                run_slot()
                        paS = phAt.tile([128, 512], bf, name="paS", tag="paS")
                        nc.vector.tensor_add(paS[:], paP[:], paD[:])
                        avs = phAt.tile([128, 512], f32, name="avs", tag="avs")
                        nc.scalar.copy(avs[:], av[:])
                        finA, rzh = make_finA(paS)
                        pending_fins.append(finA)
                        pending_fins.append(make_finB(rzh, avs, afv, h))
                    last = lqc == NLC - 1
                    for ltl in range(4):
                        for dc in range(D // 512):
                            pending_ops.append(make_opchunk(
                                afv, lqc, ltl, dc,
                                on_act=last and (ltl + dc) % 2 == 1))
                while pending_fins or pending_ops:
                    run_slot()

    nc.finalize()
    return nc


def _get_program(reps=1):
    if reps not in _PROG:
        _PROG[reps] = _build_program(reps=reps)
    return _PROG[reps]


_PERM = np.concatenate([np.arange(0, DH, 2), np.arange(1, DH, 2)])


def make_core_inputs(inputs: dict) -> list:
    hs = np.asarray(inputs["hidden_states"], np.float32)
    enc = np.asarray(inputs["encoder_hidden_states"], np.float32)
    mask = np.asarray(inputs["attention_mask"]).astype(bool)
    emb = np.asarray(inputs["image_rotary_emb"], np.float32)
    wqkv = np.asarray(inputs["w_img_qkv"], np.float32).reshape(D, 3, H, DH)
    wtkv = np.asarray(inputs["w_txt_kv"], np.float32).reshape(D, 2, H, DH)
    wout = np.asarray(inputs["w_out"], np.float32).reshape(H, DH, D)
    g_q = np.asarray(inputs["g_q"], np.float32)
    g_k = np.asarray(inputs["g_k"], np.float32)
    g_ak = np.asarray(inputs["g_added_k"], np.float32)

    def tables(F, g):
        # F: [L, 64, 2, 2]; permuted layout: part p<64 -> dim 2p, 64+p -> 2p+1
        # dst = tabA * ev + tabB * swap64(ev)
        ge, go = g[0::2], g[1::2]
        tabA = np.concatenate([(F[:, :, 0, 0] * ge[None, :]).T,
                               (F[:, :, 1, 1] * go[None, :]).T], axis=0)
        tabB = np.concatenate([(F[:, :, 0, 1] * go[None, :]).T,
                               (F[:, :, 1, 0] * ge[None, :]).T], axis=0)
        return np.stack([tabA, tabB], axis=1).astype(bf16)  # [128, 2, L]

    in_maps = []
    for c in range(NCORES):
        b, g = divmod(c, 4)
        hsel = slice(g * HPC, (g + 1) * HPC)
        F = emb[b, 0]
        wq = wqkv[:, 0, hsel, :][:, :, _PERM].reshape(D, HPC * DH)
        wk = wqkv[:, 1, hsel, :][:, :, _PERM].reshape(D, HPC * DH)
        wv = wqkv[:, 2, hsel, :].reshape(D, HPC * DH)
        wtk = wtkv[:, 0, hsel, :][:, :, _PERM].reshape(D, HPC * DH)
        wtv = wtkv[:, 1, hsel, :].reshape(D, HPC * DH)
        wo = wout[hsel].reshape(HPC * DH, D)
        in_maps.append({
            "xt": np.ascontiguousarray(hs[b].T).astype(bf16),
            "et": np.ascontiguousarray((enc[b] * mask[b][:, None]).T).astype(bf16),
            "wq": np.ascontiguousarray(wq).astype(bf16),
            "wk": np.ascontiguousarray(wk).astype(bf16),
            "wv": np.ascontiguousarray(wv).astype(bf16),
            "wtk": np.ascontiguousarray(wtk).astype(bf16),
            "wtv": np.ascontiguousarray(wtv).astype(bf16),
            "wo": np.ascontiguousarray(wo).astype(bf16),
            "tq": tables(F, g_q),
            "tk": tables(F, g_k),
            "gtk": g_ak[_PERM].reshape(DH, 1).astype(np.float32),
            "nm": np.array([[-(float(L_TXT) - float(mask[b].sum()))]], np.float32),
        })
    return in_maps


def run_cores(in_maps, trace=False, tmpdir=None):
    from concourse.bass_utils import run_bass_kernel_spmd
    nc = _get_program()
    return run_bass_kernel_spmd(nc, in_maps, list(range(NCORES)),
                                trace=trace, tmpdir=tmpdir)


def kernel(**inputs) -> np.ndarray:
    in_maps = make_core_inputs(inputs)
    res = run_cores(in_maps)
    out = np.zeros((B, L_IMG, D), np.float32)
    for c in range(NCORES):
        b = c // 4
        out[b] += np.asarray(res.results[c]["out"], np.float32)
    return out
